# revision 1
# baseline (speedup 1.0000x reference)
"""Local-window (banded) multi-head attention on 8 Trainium2 NeuronCores.

Problem: x[L=2048, B=8, D=512], Wqkv[1536, 512], Wout[512, 512], bout[512].
  qkv = x @ Wqkv.T ; per-head banded attention (|i-j| <= 64, window 129);
  out = attn_out @ Wout.T + bout.

Sharding: batch B=8 across the 8 cores (data parallel). Each core runs the
full pipeline for one batch element. Inputs are pre-transposed host-side so
all device matmuls contract over the partition dimension:

  xT[d, l], WqkvT[d, c], WoutT[d', c] in SBUF; scores are computed
  TRANSPOSED (scoresT[m, l] = K @ Q^T) so that softmax normalization and
  the P@V contraction both happen along the partition (m) axis with zero
  on-chip transposes. The softmax denominator comes for free from an
  appended ones-column in V; normalization uses a tiny K=1 broadcast matmul.

Matmuls run in float32r (fp32 storage, fast PE path, N>=256).
"""

import os
import sys

import numpy as np

if "/opt/trn_rl_repo" not in sys.path:
    sys.path.insert(0, "/opt/trn_rl_repo")

L, B, D, H, DH = 2048, 8, 512, 8, 64
WIN, PAD = 129, 64
C3 = 3 * D  # 1536
NK = D // 128  # 4 contraction tiles
NLT = L // 128  # 16 l-tiles
NCH = L // 256  # 8 attention l-chunks of 256
HEAD_STRIDE = DH + 1  # 65: V columns per head incl. ones column

_NC_CACHE = {}


def _build_nc():
    from concourse import bacc, mybir, tile

    f32 = mybir.dt.float32
    f32r = mybir.dt.float32r
    Exp = mybir.ActivationFunctionType.Exp
    is_ge = mybir.AluOpType.is_ge

    nc = bacc.Bacc(None, target_bir_lowering=False)

    xT_d = nc.dram_tensor("xT", [D, L], f32r, kind="ExternalInput")
    wqkvT_d = nc.dram_tensor("wqkvT", [D, C3], f32r, kind="ExternalInput")
    woutT_d = nc.dram_tensor("woutT", [D, D], f32r, kind="ExternalInput")
    bout_d = nc.dram_tensor("bout", [D], f32, kind="ExternalInput")
    zeros_d = nc.dram_tensor("zeros_c", [128, 640], f32r, kind="ExternalInput")
    onesc_d = nc.dram_tensor("ones_c", [128, 8], f32r, kind="ExternalInput")
    y_d = nc.dram_tensor("y", [L, D], f32, kind="ExternalOutput")

    import concourse.bass as bass

    KTW = 64 + L + 64  # K^T cols: zero-pad both sides

    with tile.TileContext(nc) as tc, nc.allow_low_precision(
        reason="float32r tiles feed the PE fast path; accumulation stays fp32 in PSUM"
    ):
        with (
            tc.tile_pool(name="pers", bufs=1) as pers,
            tc.tile_pool(name="ps", bufs=1, space="PSUM") as ps,
        ):
            # ---- persistent SBUF tensors (everything stays resident) ----
            xT = [pers.tile([128, L], f32r, name=f"xT{k}", tag=f"xT{k}") for k in range(NK)]
            wqkvT = [
                pers.tile([128, C3], f32r, name=f"wqkvT{k}", tag=f"wqkvT{k}")
                for k in range(NK)
            ]
            woutT = [
                pers.tile([128, D], f32r, name=f"woutT{k}", tag=f"woutT{k}")
                for k in range(NK)
            ]
            boutb = pers.tile([128, D], f32, name="boutb", tag="boutb")
            ones1 = pers.tile([1, DH], f32r, name="ones1", tag="ones1")
            QT = [pers.tile([128, L], f32r, name=f"QT{t}", tag=f"QT{t}") for t in range(NK)]
            KT = [
                pers.tile([128, KTW], f32r, name=f"KT{t}", tag=f"KT{t}")
                for t in range(NK)
            ]
            Vs = [
                pers.tile([128, H * HEAD_STRIDE], f32r, name=f"Vs{j}", tag=f"Vs{j}")
                for j in range(NLT + 1)
            ]
            # per-chunk normalized O^T buffers come from a rotating pool
            # (allocated per (t, ch) inside the loop)

            def mm(out, lhsT, rhs, start, stop):
                nc.tensor.matmul(out, lhsT, rhs, start=start, stop=stop)

            # ---- input DMAs: column-sliced + interleaved across both HWDGE
            # rings so the first projection groups unblock within ~3us ----
            for ch in range(4):
                cs = slice(ch * 512, (ch + 1) * 512)
                for k in range(NK):
                    eng = nc.sync if (k + ch) % 2 == 0 else nc.scalar
                    eng.dma_start(out=xT[k][:, cs], in_=xT_d[k * 128 : (k + 1) * 128, cs])
                # wqkvT thirds in Q, K, V priority order per round
                third = [0, D, 2 * D, None][ch]
                if third is not None:
                    ws = slice(third, third + 512)
                    for k in range(NK):
                        eng = nc.scalar if (k + ch) % 2 == 0 else nc.sync
                        eng.dma_start(
                            out=wqkvT[k][:, ws], in_=wqkvT_d[k * 128 : (k + 1) * 128, ws]
                        )
            for k in range(NK):
                nc.sync.dma_start(
                    out=woutT[k][:], in_=woutT_d[k * 128 : (k + 1) * 128, :]
                )
            bout_ap = bout_d[:]
            bout_bcast = bass.AP(
                tensor=bout_ap.tensor, offset=bout_ap.offset, ap=[[0, 128], [1, D]]
            )
            nc.gpsimd.dma_start(out=boutb[:], in_=bout_bcast)
            nc.gpsimd.dma_start(
                out=ones1[:], in_=onesc_d[0:DH, 0:1].rearrange("a b -> b a")
            )
            # zero K^T left pad and the out-of-range halves of the shifted V
            for t in range(NK):
                nc.sync.dma_start(out=KT[t][:, 0:64], in_=zeros_d[:, 0:64])
                nc.sync.dma_start(
                    out=KT[t][:, 64 + L : KTW], in_=zeros_d[:, 0:64]
                )
            nc.sync.dma_start(
                out=Vs[0][0:64, :], in_=zeros_d[0:64, 0 : H * HEAD_STRIDE]
            )
            nc.sync.dma_start(
                out=Vs[NLT][64:128, :], in_=zeros_d[0:64, 0 : H * HEAD_STRIDE]
            )
            # ones column for every head slot (softmax denom via PV matmul)
            for j in range(NLT + 1):
                vcol = Vs[j].rearrange("p (h e) -> p h e", e=HEAD_STRIDE)
                nc.gpsimd.dma_start(
                    out=vcol[:, :, DH : DH + 1],
                    in_=onesc_d[:].rearrange("p (h e) -> p h e", e=1),
                )

            # ---- phase B: projections, interleaved so attention unblocks
            # early: Q/K chunk round first, then a slice of V tiles ----
            def b1_vproj(lts):
                for lt in lts:
                    vp = ps.tile([128, D], f32, name=f"vp{lt}", tag="big", bufs=2)
                    for k in range(NK):
                        mm(
                            vp[:],
                            xT[k][:, lt * 128 : (lt + 1) * 128],
                            wqkvT[k][:, 2 * D : 3 * D],
                            start=(k == 0),
                            stop=(k == NK - 1),
                        )
                    src_v = vp.rearrange("p (h e) -> p h e", e=DH)
                    dlo = Vs[lt][64:128, :].rearrange("p (h e) -> p h e", e=HEAD_STRIDE)
                    dhi = Vs[lt + 1][0:64, :].rearrange(
                        "p (h e) -> p h e", e=HEAD_STRIDE
                    )
                    nc.scalar.copy(out=dlo[:, :, 0:DH], in_=src_v[0:64])
                    nc.vector.tensor_copy(out=dhi[:, :, 0:DH], in_=src_v[64:128])

            for ch in range(4):  # l-chunks of 512
                for t in range(NK):
                    for which in range(2):  # 0 -> Q tile t, 1 -> K tile t
                        c0 = which * D + t * 128
                        qp = ps.tile(
                            [128, 512], f32, name=f"qp{t}_{which}_{ch}",
                            tag="big", bufs=2,
                        )
                        for k in range(NK):
                            mm(
                                qp[:],
                                wqkvT[k][:, c0 : c0 + 128],
                                xT[k][:, ch * 512 : (ch + 1) * 512],
                                start=(k == 0),
                                stop=(k == NK - 1),
                            )
                        if which == 0:
                            dest = QT[t][:, ch * 512 : (ch + 1) * 512]
                        else:
                            dest = KT[t][:, 64 + ch * 512 : 64 + (ch + 1) * 512]
                        nc.vector.tensor_copy(out=dest, in_=qp[:])
                b1_vproj(range(4 * ch, 4 * ch + 4))

            # ---- phase C+D: banded attention + fused output projection ----
            def emit_D(dch, bufs):
                # output projection for chunk dch's two l-tiles
                for half in range(2):
                    lt = 2 * dch + half
                    yp = ps.tile([128, D], f32, name=f"yp{lt}", tag="big", bufs=2)
                    for k in range(NK):
                        mm(
                            yp[:],
                            bufs[k][:, half * 128 : (half + 1) * 128],
                            woutT[k][:],
                            start=(k == 0),
                            stop=(k == NK - 1),
                        )
                    ysb = pers.tile([128, D], f32, name=f"ysb{lt}", tag="ysb", bufs=2)
                    nc.vector.tensor_add(out=ysb[:], in0=yp[:], in1=boutb[:])
                    nc.sync.dma_start(out=y_d[lt * 128 : (lt + 1) * 128, :], in_=ysb[:])

            prev_otc = None
            for ch in range(NCH):
                cur_otc = []
                for t in range(NK):
                    if t == 2 and prev_otc is not None:
                        emit_D(ch - 1, prev_otc)
                    otc = None
                    otmp = None
                    for hh in range(2):
                        h = 2 * t + hh
                        p0 = hh * 64
                        qsl = QT[t][p0 : p0 + 64, ch * 256 : (ch + 1) * 256]
                        # fused scores psum: 3 m-tiles side by side (2 banks)
                        scp = ps.tile(
                            [128, 768], f32, name=f"sc{h}_{ch}", tag="sc", bufs=2
                        )
                        for r in range(3):
                            kcol = 256 * ch + 128 * r  # into padded KT columns
                            mm(
                                scp[:, 256 * r : 256 * (r + 1)],
                                KT[t][p0 : p0 + 64, kcol : kcol + 128],
                                qsl,
                                start=True,
                                stop=True,
                            )
                        pt = wk_tile = pers.tile(
                            [128, 768], f32r, name=f"pt{h}_{ch}", tag="pt", bufs=4
                        )
                        nc.scalar.activation(
                            out=pt[:], in_=scp[:], func=Exp, scale=0.125
                        )
                        # band mask per m-tile r: keep iff 0 <= (128r + p) - f <= 128
                        # fused as two 2-block selects over the 768-wide tile
                        pAB = pt[:, 0:512].rearrange("p (b f) -> p b f", f=256)
                        pBC = pt[:, 256:768].rearrange("p (b f) -> p b f", f=256)
                        nc.gpsimd.affine_select(
                            out=pAB, in_=pAB, compare_op=is_ge, fill=0.0,
                            base=0, pattern=[[128, 2], [-1, 256]],
                            channel_multiplier=1,
                        )
                        nc.gpsimd.affine_select(
                            out=pBC, in_=pBC, compare_op=is_ge, fill=0.0,
                            base=0, pattern=[[-128, 2], [1, 256]],
                            channel_multiplier=-1,
                        )
                        if ch == 0:  # global key index p-64 must be >= 0 (r0)
                            p_r0 = pt[:, 0:256]
                            nc.gpsimd.affine_select(
                                out=p_r0, in_=p_r0, compare_op=is_ge, fill=0.0,
                                base=-64, pattern=[[0, 256]], channel_multiplier=1,
                            )
                        if ch == NCH - 1:  # global key index 1984+p < L (r2)
                            p_r2 = pt[:, 512:768]
                            nc.gpsimd.affine_select(
                                out=p_r2, in_=p_r2, compare_op=is_ge, fill=0.0,
                                base=63, pattern=[[0, 256]], channel_multiplier=-1,
                            )
                        # P~ @ V (transposed): O'[d, l] with denom in row DH.
                        # Both heads share one PSUM bank (disjoint column halves;
                        # PE executes matmuls in program order, so hh=1's
                        # start=True bank-clear cannot interleave hh=0's group).
                        if hh == 0:
                            op = ps.tile(
                                [DH + 1, 512], f32, name=f"op{t}_{ch}", tag="o",
                                bufs=2,
                            )
                        for r in range(3):
                            vsl = Vs[2 * ch + r][
                                :, h * HEAD_STRIDE : (h + 1) * HEAD_STRIDE
                            ]
                            mm(
                                op[:, 256 * hh : 256 * (hh + 1)],
                                vsl,
                                pt[:, 256 * r : 256 * (r + 1)],
                                start=(r == 0),
                                stop=(r == 2),
                            )
                        if hh == 1:
                            otmp = pers.tile(
                                [DH + 1, 512], f32, name=f"otm{t}_{ch}", tag="otmp",
                                bufs=4,
                            )
                            nc.scalar.copy(out=otmp[:], in_=op[:])
                    # decoupled normalization for the head pair
                    rbp = ps.tile([DH, 512], f32, name=f"rbp{t}_{ch}", tag="big", bufs=2)
                    rr = pers.tile([1, 512], f32r, name=f"rr{t}_{ch}", tag="rr", bufs=2)
                    nc.vector.reciprocal(out=rr[:], in_=otmp[DH : DH + 1, :])
                    for hh in range(2):
                        mm(rbp[:, 256 * hh : 256 * (hh + 1)], ones1[:],
                           rr[:, 256 * hh : 256 * (hh + 1)], start=True, stop=True)
                    otc = pers.tile(
                        [128, 256], f32r, name=f"OTc{t}_{ch}", tag=f"OTc{t}", bufs=2
                    )
                    for hh in range(2):
                        nc.vector.tensor_mul(
                            out=otc[64 * hh : 64 * (hh + 1), :],
                            in0=otmp[0:DH, 256 * hh : 256 * (hh + 1)],
                            in1=rbp[:, 256 * hh : 256 * (hh + 1)],
                        )
                    cur_otc.append(otc)
                prev_otc = cur_otc
            emit_D(NCH - 1, prev_otc)

    nc.compile()
    return nc


def get_nc():
    if "nc" not in _NC_CACHE:
        _NC_CACHE["nc"] = _build_nc()
    return _NC_CACHE["nc"]


def make_core_inputs(x, Wqkv, Wout, bout):
    """Host-side shard + layout prep: per-core transposed views."""
    x = np.asarray(x, dtype=np.float32)
    wqkvT = np.ascontiguousarray(np.asarray(Wqkv, dtype=np.float32).T)
    woutT = np.ascontiguousarray(np.asarray(Wout, dtype=np.float32).T)
    bout = np.ascontiguousarray(np.asarray(bout, dtype=np.float32))
    in_maps = []
    for b in range(B):
        in_maps.append(
            {
                "xT": np.ascontiguousarray(x[:, b, :].T),
                "wqkvT": wqkvT,
                "woutT": woutT,
                "bout": bout,
                "zeros_c": np.zeros((128, 640), dtype=np.float32),
                "ones_c": np.ones((128, 8), dtype=np.float32),
            }
        )
    return in_maps


def kernel(x, Wqkv, Wout, bout):
    from concourse.bass_utils import run_bass_kernel_spmd

    nc = get_nc()
    in_maps = make_core_inputs(x, Wqkv, Wout, bout)
    res = run_bass_kernel_spmd(nc, in_maps, core_ids=list(range(B)))
    out = np.empty((L, B, D), dtype=np.float32)
    for b in range(B):
        out[:, b, :] = res.results[b]["y"]
    return out



# revision 27
# speedup vs baseline: 1.4266x; 1.4266x over previous
"""Local-window (banded) multi-head attention on 8 Trainium2 NeuronCores.

Problem: x[L=2048, B=8, D=512], Wqkv[1536, 512], Wout[512, 512], bout[512].
  qkv = x @ Wqkv.T ; per-head banded attention (|i-j| <= 64, window 129);
  out = attn_out @ Wout.T + bout.

Sharding: batch B=8 across the 8 cores (data parallel). Each core runs the
full pipeline for one batch element, all matmul operands in bf16 (fp32 PSUM
accumulation):

  - 128-query chunks x 256-key windows (2 aligned key tiles against a
    64-padded K^T) minimize banded-score work.
  - P@V runs q-major (P^T stationary, V moving 64 cols) so the softmax
    denominator is a per-partition column: normalization is one reciprocal +
    one broadcast multiply per chunk, no PE broadcast matmul.
  - A single batched DMA-transpose per chunk returns normalized output to
    d-major for the fused output projection.
  - The band mask is three precomputed bf16 tiles applied by tensor multiply.
  - QKV projection matmuls are interleaved into the attention loop as PE
    filler to hide the exp->mask->PV dependency chain.
"""

import sys

import numpy as np

if "/opt/trn_rl_repo" not in sys.path:
    sys.path.insert(0, "/opt/trn_rl_repo")

L, B, D, H, DH = 2048, 8, 512, 8, 64
WIN, PAD = 129, 64
C3 = 3 * D
NK = D // 128  # 4 contraction tiles
NCH = L // 128  # 16 query chunks of 128
KTW = 64 + L + 64  # padded K^T columns (col = key + 64)

_NC_CACHE = {}


class _SkipRest(Exception):
    pass


def _build_nc(n_ch=NCH, stage=99):
    from concourse import bacc, mybir, tile
    import concourse.bass as bass

    f32 = mybir.dt.float32
    bf16 = mybir.dt.bfloat16
    Exp = mybir.ActivationFunctionType.Exp
    is_ge = mybir.AluOpType.is_ge

    nc = bacc.Bacc(None, target_bir_lowering=False)

    xT_d = nc.dram_tensor("xT", [D, L], bf16, kind="ExternalInput")
    wqkvT_d = nc.dram_tensor("wqkvT", [D, C3], bf16, kind="ExternalInput")
    woutT_d = nc.dram_tensor("woutT", [D, D], bf16, kind="ExternalInput")
    bout_d = nc.dram_tensor("bout", [D], f32, kind="ExternalInput")
    y_d = nc.dram_tensor("y", [L, D], f32, kind="ExternalOutput")

    with tile.TileContext(nc) as tc, nc.allow_low_precision(
        reason="bf16 operands feed the PE; accumulation stays fp32 in PSUM"
    ):
        with (
            tc.tile_pool(name="pers", bufs=1) as pers,
            tc.tile_pool(name="ps", bufs=1, space="PSUM") as ps,
        ):
            # ---- persistent SBUF tensors ----
            xTs = [pers.tile([128, L], bf16, name=f"xT{k}", tag=f"xT{k}") for k in range(NK)]
            wqs = [
                pers.tile([128, C3], bf16, name=f"wq{k}", tag=f"wq{k}") for k in range(NK)
            ]
            wos = [
                pers.tile([128, D], bf16, name=f"wo{k}", tag=f"wo{k}") for k in range(NK)
            ]
            boutb = pers.tile([128, D], f32, name="boutb", tag="boutb")
            onesc = pers.tile([128, 1], bf16, name="onesc", tag="onesc")
            QT = [pers.tile([128, L], bf16, name=f"QT{t}", tag=f"QT{t}") for t in range(NK)]
            KT = [
                pers.tile([128, KTW], bf16, name=f"KT{t}", tag=f"KT{t}") for t in range(NK)
            ]
            Vs = [
                pers.tile([128, D], bf16, name=f"Vs{j}", tag=f"Vs{j}")
                for j in range(NCH + 1)
            ]
            Mmid = pers.tile([128, 512], bf16, name="Mmid", tag="Mmid")
            Mfirst = pers.tile([128, 512], bf16, name="Mfirst", tag="Mfirst")
            Mlast = pers.tile([128, 512], bf16, name="Mlast", tag="Mlast")

            def mm(out, lhsT, rhs, start, stop):
                nc.tensor.matmul(out, lhsT, rhs, start=start, stop=stop)

            # ---- input DMAs: interleave SP/Act rings, first-needed first ----
            dma_i = 0

            def dma(dst, src):
                nonlocal dma_i
                eng = nc.sync if dma_i % 2 == 0 else nc.scalar
                dma_i += 1
                eng.dma_start(out=dst, in_=src)

            # round 0: x window 0 + Wq third (unblocks first Q-projections)
            for k in range(NK):
                dma(xTs[k][:, 0:512], xT_d[k * 128 : (k + 1) * 128, 0:512])
                dma(wqs[k][:, 0:512], wqkvT_d[k * 128 : (k + 1) * 128, 0:512])
            # round 1: Wk + Wv thirds
            for k in range(NK):
                dma(wqs[k][:, 512:1024], wqkvT_d[k * 128 : (k + 1) * 128, 512:1024])
                dma(wqs[k][:, 1024:1536], wqkvT_d[k * 128 : (k + 1) * 128, 1024:1536])
            # rounds 2-3: x windows 1-3, then Wout
            for w in range(1, 4):
                for k in range(NK):
                    dma(
                        xTs[k][:, 512 * w : 512 * (w + 1)],
                        xT_d[k * 128 : (k + 1) * 128, 512 * w : 512 * (w + 1)],
                    )
            for k in range(NK):
                dma(wos[k][:], woutT_d[k * 128 : (k + 1) * 128, :])
            bout_ap = bout_d[:]
            bout_bcast = bass.AP(
                tensor=bout_ap.tensor, offset=bout_ap.offset, ap=[[0, 128], [1, D]]
            )
            nc.gpsimd.dma_start(out=boutb[:], in_=bout_bcast)

            # ---- Pool-side constant init (overlaps the DMA wait) ----
            nc.gpsimd.memset(onesc[:], 1.0)
            for t in range(NK):
                nc.gpsimd.memset(KT[t][:, 0:64], 0.0)
                nc.gpsimd.memset(KT[t][:, 64 + L : KTW], 0.0)
            nc.gpsimd.memset(Vs[0][0:64, :], 0.0)
            nc.gpsimd.memset(Vs[NCH][64:128, :], 0.0)
            # band masks: blocks [r0 r1 r0 r1]; r0 keeps p>=c, r1 keeps p<=c
            nc.gpsimd.memset(Mmid[:], 1.0)
            m3 = Mmid[:].rearrange("p (b c) -> p b c", c=256)
            nc.gpsimd.affine_select(
                out=m3[:, :, 0:128], in_=m3[:, :, 0:128], compare_op=is_ge,
                fill=0.0, base=0, pattern=[[0, 2], [-1, 128]], channel_multiplier=1,
            )
            nc.gpsimd.affine_select(
                out=m3[:, :, 128:256], in_=m3[:, :, 128:256], compare_op=is_ge,
                fill=0.0, base=0, pattern=[[0, 2], [1, 128]], channel_multiplier=-1,
            )
            nc.gpsimd.tensor_copy(out=Mfirst[:], in_=Mmid[:])
            f3 = Mfirst[:].rearrange("p (b c) -> p b c", c=256)
            nc.gpsimd.affine_select(  # ch 0, r0: key = p - 64 must be >= 0
                out=f3[:, :, 0:128], in_=f3[:, :, 0:128], compare_op=is_ge,
                fill=0.0, base=-64, pattern=[[0, 2], [0, 128]], channel_multiplier=1,
            )
            nc.gpsimd.tensor_copy(out=Mlast[:], in_=Mmid[:])
            l3 = Mlast[:].rearrange("p (b c) -> p b c", c=256)
            nc.gpsimd.affine_select(  # ch 15, r1: key = 1984 + p must be < 2048
                out=l3[:, :, 128:256], in_=l3[:, :, 128:256], compare_op=is_ge,
                fill=0.0, base=63, pattern=[[0, 2], [0, 128]], channel_multiplier=-1,
            )

            # ---- projection emitters (PE filler during attention) ----
            copy_i = 0

            def psum_copy(dst, src):
                # drain PSUM -> SBUF (casts f32 -> bf16); alternate Act/DVE
                nonlocal copy_i
                copy_i += 1
                if copy_i % 8 < 3:
                    nc.scalar.copy(out=dst, in_=src)
                else:
                    nc.vector.tensor_copy(out=dst, in_=src)

            def emit_qk(t, which, w):
                # Q (which=0) / K (which=1) projection, 512-query window w
                c0 = which * D + t * 128
                qp = ps.tile([128, 512], f32, name=f"qp{t}_{which}_{w}", tag="big", bufs=2)
                for k in range(NK):
                    mm(
                        qp[:],
                        wqs[k][:, c0 : c0 + 128],
                        xTs[k][:, 512 * w : 512 * (w + 1)],
                        start=(k == 0),
                        stop=(k == NK - 1),
                    )
                if which == 0:
                    dest = QT[t][:, 512 * w : 512 * (w + 1)]
                else:
                    dest = KT[t][:, 64 + 512 * w : 64 + 512 * (w + 1)]
                psum_copy(dest, qp[:])

            def emit_v(lt):
                vp = ps.tile([128, 512], f32, name=f"vp{lt}", tag="big", bufs=2)
                for k in range(NK):
                    mm(
                        vp[:],
                        xTs[k][:, lt * 128 : (lt + 1) * 128],
                        wqs[k][:, 2 * D : 3 * D],
                        start=(k == 0),
                        stop=(k == NK - 1),
                    )
                psum_copy(Vs[lt][64:128, :], vp[0:64, :])
                psum_copy(Vs[lt + 1][0:64, :], vp[64:128, :])

            # fill-group schedule: prologue covers attention chunks 0-2;
            # window w of Q/K (8 groups) is spread over chunks 4(w-1)..4(w-1)+2
            fill_groups = {ch: [] for ch in range(NCH)}
            for w in range(1, 4):
                base = 4 * (w - 1)
                sched = [3, 3, 2]
                gi = 0
                for off, cnt in enumerate(sched):
                    for _ in range(cnt):
                        t, which = gi % 4, gi // 4
                        fill_groups[base + off].append(
                            (emit_qk, (t, which, w))
                        )
                        gi += 1
            for lt in range(2, NCH):
                fill_groups[lt - 2].append((emit_v, (lt,)))

            # ---- prologue projections: Q/K window 0 + V tiles 0,1 ----
            if stage >= 2:
                for which in range(2):
                    for t in range(NK):
                        emit_qk(t, which, 0)
                emit_v(0)
                emit_v(1)

            # ---- main loop: banded attention + interleaved proj + out-proj ----
            otts = [None] * NCH

            def emit_outproj(ch):
                yp = ps.tile([128, 512], f32, name=f"yp{ch}", tag="big", bufs=2)
                for t in range(NK):
                    mm(
                        yp[:],
                        otts[ch][:, t * 128 : (t + 1) * 128],
                        wos[t][:],
                        start=(t == 0),
                        stop=(t == NK - 1),
                    )
                ysb = pers.tile([128, D], f32, name=f"ysb{ch}", tag="ysb", bufs=2)
                nc.vector.tensor_add(out=ysb[:], in0=yp[:], in1=boutb[:])
                nc.sync.dma_start(out=y_d[ch * 128 : (ch + 1) * 128, :], in_=ysb[:])

            for ch in range(n_ch if stage >= 3 else 0):
                fills = fill_groups[ch]
                fi = 0

                def fill(n):
                    nonlocal fi
                    for _ in range(min(n, len(fills) - fi)):
                        fn, args = fills[fi]
                        fn(*args)
                        fi += 1

                mask = Mfirst if ch == 0 else (Mlast if ch == NCH - 1 else Mmid)
                # scores in t-pair groups: one [128,1024] psum tile spans two
                # banks; all p0=0 blocks fill bank A, p0=64 blocks bank B (the
                # PE wedges if the operand partition offset switches while
                # targeting the same PSUM bank).
                pts = []
                for pg in range(2):
                    scp = ps.tile(
                        [128, 1024], f32, name=f"sc{ch}_{pg}", tag="sc", bufs=2
                    )
                    for hh in range(2):
                        p0 = 64 * hh
                        for ti in range(2):
                            t = 2 * pg + ti
                            for r in range(2):
                                blk = 4 * hh + 2 * ti + r
                                mm(
                                    scp[:, blk * 128 : (blk + 1) * 128],
                                    KT[t][
                                        p0 : p0 + 64,
                                        128 * (ch + r) : 128 * (ch + r + 1),
                                    ],
                                    QT[t][p0 : p0 + 64, 128 * ch : 128 * (ch + 1)],
                                    start=True,
                                    stop=True,
                                )
                    if stage == 31:
                        continue
                    pt = pers.tile(
                        [128, 1024], bf16, name=f"pt{ch}_{pg}", tag="pt", bufs=3
                    )
                    nc.scalar.activation(out=pt[:], in_=scp[:], func=Exp, scale=0.125)
                    if stage == 32:
                        pts.append(pt)
                        continue
                    nc.gpsimd.tensor_mul(out=pt[:, 0:512], in0=pt[:, 0:512], in1=mask[:])
                    nc.gpsimd.tensor_mul(
                        out=pt[:, 512:1024], in0=pt[:, 512:1024], in1=mask[:]
                    )
                    pts.append(pt)

                fill(1)
                if stage < 4 or stage in (31, 32):
                    continue
                opc = ps.tile([128, 512], f32, name=f"op{ch}", tag="op", bufs=1)
                dnc = ps.tile([128, 8], f32, name=f"dn{ch}", tag="dn", bufs=1)
                for t in range(NK):
                    pt = pts[t // 2]
                    for hh in range(2):
                        h = 2 * t + hh
                        for r in range(2):
                            b = 4 * hh + 2 * (t % 2) + r
                            blk = pt[:, b * 128 : (b + 1) * 128]
                            mm(
                                dnc[:, h : h + 1], blk, onesc[:],
                                start=(r == 0), stop=(r == 1),
                            )
                            mm(
                                opc[:, 64 * h : 64 * (h + 1)],
                                blk,
                                Vs[ch + r][:, 64 * h : 64 * (h + 1)],
                                start=(r == 0),
                                stop=(r == 1),
                            )
                    if t < NK - 1:
                        fill(1)

                # normalization: per-query reciprocal, broadcast along d, mult
                if stage < 5:
                    continue
                rr = pers.tile([128, 8], f32, name=f"rr{ch}", tag="rr", bufs=2)
                nc.vector.reciprocal(out=rr[:], in_=dnc[:])
                rb = pers.tile([128, 512], f32, name=f"rb{ch}", tag="rb", bufs=2)
                rr_ap = rr[:]
                rr_b = bass.AP(
                    tensor=rr_ap.tensor,
                    offset=rr_ap.offset,
                    ap=[list(rr_ap.ap[0]), [1, 8], [0, 64]],
                )
                nc.gpsimd.tensor_copy(
                    out=rb[:].rearrange("p (h e) -> p h e", e=64), in_=rr_b
                )
                otq = pers.tile([128, 512], bf16, name=f"otq{ch}", tag="otq", bufs=2)
                nc.vector.tensor_mul(out=otq[:], in0=opc[:], in1=rb[:])
                if stage < 6:
                    continue
                # batched transpose back to d-major: ott[d, t, q] = otq[q, t, d]
                ott = pers.tile([128, 512], bf16, name=f"ott{ch}", tag="ott", bufs=3)
                otts[ch] = ott
                nc.sync.dma_start_transpose(
                    out=ott[:].rearrange("p (t q) -> p t q", q=128), in_=otq[:]
                )
                fill(len(fills))
                if stage < 7:
                    continue
                if ch >= 1:
                    emit_outproj(ch - 1)
            if stage >= 7 and stage not in (31, 32):
                emit_outproj(n_ch - 1)

    nc.compile()
    return nc


def get_nc():
    if "nc" not in _NC_CACHE:
        _NC_CACHE["nc"] = _build_nc()
    return _NC_CACHE["nc"]


def make_core_inputs(x, Wqkv, Wout, bout):
    """Host-side shard + layout prep: per-core transposed bf16 views."""
    from concourse import mybir

    bf16 = mybir.dt.np(mybir.dt.bfloat16)
    x = np.asarray(x, dtype=np.float32)
    wqkvT = np.ascontiguousarray(np.asarray(Wqkv, dtype=np.float32).T).astype(bf16)
    woutT = np.ascontiguousarray(np.asarray(Wout, dtype=np.float32).T).astype(bf16)
    bout = np.ascontiguousarray(np.asarray(bout, dtype=np.float32))
    in_maps = []
    for b in range(B):
        in_maps.append(
            {
                "xT": np.ascontiguousarray(x[:, b, :].T).astype(bf16),
                "wqkvT": wqkvT,
                "woutT": woutT,
                "bout": bout,
            }
        )
    return in_maps


def kernel(x, Wqkv, Wout, bout):
    from concourse.bass_utils import run_bass_kernel_spmd

    nc = get_nc()
    in_maps = make_core_inputs(x, Wqkv, Wout, bout)
    res = run_bass_kernel_spmd(nc, in_maps, core_ids=list(range(B)))
    out = np.empty((L, B, D), dtype=np.float32)
    for b in range(B):
        out[:, b, :] = res.results[b]["y"]
    return out


# revision 50
# speedup vs baseline: 1.4799x; 1.0374x over previous
"""Local-window (banded) multi-head attention on 8 Trainium2 NeuronCores.

Problem: x[L=2048, B=8, D=512], Wqkv[1536, 512], Wout[512, 512], bout[512].
  qkv = x @ Wqkv.T ; per-head banded attention (|i-j| <= 64, window 129);
  out = attn_out @ Wout.T + bout.

Sharding: batch B=8 across the 8 cores (data parallel). Each core runs the
full pipeline for one batch element, all matmul operands in bf16 (fp32 PSUM
accumulation):

  - 128-query chunks x 256-key windows (2 aligned key tiles against a
    64-padded K^T) minimize banded-score work.
  - P@V runs q-major (P^T stationary, V moving 64 cols) so the softmax
    denominator is a per-partition column: normalization is one reciprocal +
    one broadcast multiply per chunk, no PE broadcast matmul.
  - A single batched DMA-transpose per chunk returns normalized output to
    d-major for the fused output projection.
  - The band mask is three precomputed bf16 tiles applied by tensor multiply.
  - QKV projection matmuls are interleaved into the attention loop as PE
    filler to hide the exp->mask->PV dependency chain.
"""

import sys

import numpy as np

if "/opt/trn_rl_repo" not in sys.path:
    sys.path.insert(0, "/opt/trn_rl_repo")

L, B, D, H, DH = 2048, 8, 512, 8, 64
WIN, PAD = 129, 64
C3 = 3 * D
NK = D // 128  # 4 contraction tiles
NCH = L // 128  # 16 query chunks of 128
KTW = 64 + L + 64  # padded K^T columns (col = key + 64)

_NC_CACHE = {}


class _SkipRest(Exception):
    pass


def _build_nc(n_ch=NCH, stage=99):
    from concourse import bacc, mybir, tile
    import concourse.bass as bass

    f32 = mybir.dt.float32
    bf16 = mybir.dt.bfloat16
    Exp = mybir.ActivationFunctionType.Exp
    is_ge = mybir.AluOpType.is_ge

    nc = bacc.Bacc(None, target_bir_lowering=False)

    xT_d = nc.dram_tensor("xT", [D, L], bf16, kind="ExternalInput")
    wqkvT_d = nc.dram_tensor("wqkvT", [D, C3], bf16, kind="ExternalInput")
    woutT_d = nc.dram_tensor("woutT", [D, D], bf16, kind="ExternalInput")
    bout_d = nc.dram_tensor("bout", [D], f32, kind="ExternalInput")
    y_d = nc.dram_tensor("y", [L, D], f32, kind="ExternalOutput")

    with tile.TileContext(nc) as tc, nc.allow_low_precision(
        reason="bf16 operands feed the PE; accumulation stays fp32 in PSUM"
    ):
        with (
            tc.tile_pool(name="pers", bufs=1) as pers,
            tc.tile_pool(name="ps", bufs=1, space="PSUM") as ps,
        ):
            # ---- persistent SBUF tensors ----
            xTs = [pers.tile([128, L], bf16, name=f"xT{k}", tag=f"xT{k}") for k in range(NK)]
            wqs = [
                pers.tile([128, C3], bf16, name=f"wq{k}", tag=f"wq{k}") for k in range(NK)
            ]
            wos = [
                pers.tile([128, D], bf16, name=f"wo{k}", tag=f"wo{k}") for k in range(NK)
            ]
            boutb = pers.tile([128, D], f32, name="boutb", tag="boutb")
            onesc = pers.tile([128, 1], bf16, name="onesc", tag="onesc")
            QT = [pers.tile([128, L], bf16, name=f"QT{t}", tag=f"QT{t}") for t in range(NK)]
            KT = [
                pers.tile([128, KTW], bf16, name=f"KT{t}", tag=f"KT{t}") for t in range(NK)
            ]
            Vs = [
                pers.tile([128, D], bf16, name=f"Vs{j}", tag=f"Vs{j}")
                for j in range(NCH + 1)
            ]
            Mmid = pers.tile([128, 512], bf16, name="Mmid", tag="Mmid")
            Mfirst = pers.tile([128, 512], bf16, name="Mfirst", tag="Mfirst")
            Mlast = pers.tile([128, 512], bf16, name="Mlast", tag="Mlast")

            def mm(out, lhsT, rhs, start, stop):
                nc.tensor.matmul(out, lhsT, rhs, start=start, stop=stop)

            # ---- input DMAs: interleave SP/Act rings, first-needed first ----
            dma_i = 0

            def dma(dst, src):
                nonlocal dma_i
                eng = nc.sync if dma_i % 2 == 0 else nc.scalar
                dma_i += 1
                eng.dma_start(out=dst, in_=src)

            # fine-grained starters (both rings): x window-0 halves and Wq
            # Q-third quarters so the first projection group unblocks ~2.8us
            for k in range(NK):
                dma(xTs[k][:, 0:256], xT_d[k * 128 : (k + 1) * 128, 0:256])
                dma(wqs[k][:, 0:256], wqkvT_d[k * 128 : (k + 1) * 128, 0:256])
            for k in range(NK):
                dma(xTs[k][:, 256:512], xT_d[k * 128 : (k + 1) * 128, 256:512])
                dma(wqs[k][:, 256:512], wqkvT_d[k * 128 : (k + 1) * 128, 256:512])
            # Wk then Wv thirds (K-projection groups run before V groups)
            for k in range(NK):
                dma(wqs[k][:, 512:1024], wqkvT_d[k * 128 : (k + 1) * 128, 512:1024])
            for k in range(NK):
                dma(wqs[k][:, 1024:1536], wqkvT_d[k * 128 : (k + 1) * 128, 1024:1536])
            # coarse remainder on the SP ring only (keeps Act free for copies
            # and leaves SP clear before the per-chunk transposes start)
            for k in range(NK):
                nc.sync.dma_start(
                    out=xTs[k][:, 512:2048],
                    in_=xT_d[k * 128 : (k + 1) * 128, 512:2048],
                )
            for k in range(NK):
                nc.sync.dma_start(out=wos[k][:], in_=woutT_d[k * 128 : (k + 1) * 128, :])
            bout_ap = bout_d[:]
            bout_bcast = bass.AP(
                tensor=bout_ap.tensor, offset=bout_ap.offset, ap=[[0, 128], [1, D]]
            )
            nc.gpsimd.dma_start(out=boutb[:], in_=bout_bcast)

            # ---- Pool-side constant init (overlaps the DMA wait) ----
            nc.gpsimd.memset(onesc[:], 1.0)
            ident = pers.tile([128, 128], bf16, name="ident", tag="ident")
            nc.gpsimd.memset(ident[:], 1.0)
            nc.gpsimd.affine_select(
                out=ident[:], in_=ident[:], compare_op=mybir.AluOpType.is_equal,
                fill=0.0, base=0, pattern=[[-1, 128]], channel_multiplier=1,
            )
            ones_row = pers.tile([1, 128], bf16, name="ones_row", tag="ones_row")
            nc.gpsimd.memset(ones_row[:], 1.0)
            bout_row = pers.tile([1, D], bf16, name="bout_row", tag="bout_row")
            nc.gpsimd.tensor_copy(out=bout_row[:], in_=boutb[0:1, :])
            for t in range(NK):
                nc.gpsimd.memset(KT[t][:, 0:64], 0.0)
                nc.gpsimd.memset(KT[t][:, 64 + L : KTW], 0.0)
            nc.gpsimd.memset(Vs[0][0:64, :], 0.0)
            nc.gpsimd.memset(Vs[NCH][64:128, :], 0.0)
            # band masks: blocks [r0 r1 r0 r1]; r0 keeps p>=c, r1 keeps p<=c
            nc.gpsimd.memset(Mmid[:], 1.0)
            m3 = Mmid[:].rearrange("p (b c) -> p b c", c=256)
            nc.gpsimd.affine_select(
                out=m3[:, :, 0:128], in_=m3[:, :, 0:128], compare_op=is_ge,
                fill=0.0, base=0, pattern=[[0, 2], [-1, 128]], channel_multiplier=1,
            )
            nc.gpsimd.affine_select(
                out=m3[:, :, 128:256], in_=m3[:, :, 128:256], compare_op=is_ge,
                fill=0.0, base=0, pattern=[[0, 2], [1, 128]], channel_multiplier=-1,
            )
            nc.gpsimd.tensor_copy(out=Mfirst[:], in_=Mmid[:])
            f3 = Mfirst[:].rearrange("p (b c) -> p b c", c=256)
            nc.gpsimd.affine_select(  # ch 0, r0: key = p - 64 must be >= 0
                out=f3[:, :, 0:128], in_=f3[:, :, 0:128], compare_op=is_ge,
                fill=0.0, base=-64, pattern=[[0, 2], [0, 128]], channel_multiplier=1,
            )
            nc.gpsimd.tensor_copy(out=Mlast[:], in_=Mmid[:])
            l3 = Mlast[:].rearrange("p (b c) -> p b c", c=256)
            nc.gpsimd.affine_select(  # ch 15, r1: key = 1984 + p must be < 2048
                out=l3[:, :, 128:256], in_=l3[:, :, 128:256], compare_op=is_ge,
                fill=0.0, base=63, pattern=[[0, 2], [0, 128]], channel_multiplier=-1,
            )

            # ---- projection emitters (PE filler during attention) ----
            copy_i = 0
            prologue_copies = True  # Act's seq is busy issuing DMAs early on

            def psum_copy(dst, src):
                # drain PSUM -> SBUF (casts f32 -> bf16); alternate Act/DVE
                nonlocal copy_i
                copy_i += 1
                if prologue_copies or copy_i % 8 >= 3:
                    nc.vector.tensor_copy(out=dst, in_=src)
                else:
                    nc.scalar.copy(out=dst, in_=src)

            def emit_qk(t, which, w, half=None):
                # Q (which=0) / K (which=1) projection, 512-query window w
                # (half=0/1 emits a 256-col half-group for startup latency)
                c0 = which * D + t * 128
                l0 = 512 * w + (256 * half if half is not None else 0)
                lw = 256 if half is not None else 512
                qp = ps.tile(
                    [128, 512], f32, name=f"qp{t}_{which}_{w}_{half}", tag="big", bufs=2
                )
                for k in range(NK):
                    mm(
                        qp[:, 0:lw],
                        wqs[k][:, c0 : c0 + 128],
                        xTs[k][:, l0 : l0 + lw],
                        start=(k == 0),
                        stop=(k == NK - 1),
                    )
                base = l0 if which == 0 else 64 + l0
                dest = (QT if which == 0 else KT)[t][:, base : base + lw]
                psum_copy(dest, qp[:, 0:lw])

            def emit_v(lt):
                vp = ps.tile([128, 512], f32, name=f"vp{lt}", tag="big", bufs=2)
                for k in range(NK):
                    mm(
                        vp[:],
                        xTs[k][:, lt * 128 : (lt + 1) * 128],
                        wqs[k][:, 2 * D : 3 * D],
                        start=(k == 0),
                        stop=(k == NK - 1),
                    )
                psum_copy(Vs[lt][64:128, :], vp[0:64, :])
                psum_copy(Vs[lt + 1][0:64, :], vp[64:128, :])

            # fill-group schedule: prologue covers attention chunks 0-2;
            # window w of Q/K (8 groups) is spread over chunks 4(w-1)..4(w-1)+2
            fill_groups = {ch: [] for ch in range(NCH)}
            for w in range(1, 4):
                base = 4 * (w - 1)
                sched = [3, 3, 2]
                gi = 0
                for off, cnt in enumerate(sched):
                    for _ in range(cnt):
                        t, which = gi % 4, gi // 4
                        fill_groups[base + off].append(
                            (emit_qk, (t, which, w))
                        )
                        gi += 1
            for lt in range(2, NCH):
                fill_groups[min(lt - 2, 12)].append((emit_v, (lt,)))
            if n_ch == NCH:
                # late chunks have no projection work left; use the previous
                # chunk's output projection as the PE filler instead
                for c in (13, 14, 15):
                    fill_groups[c].append((None, (c - 1, [0, 1])))
                    fill_groups[c].append((None, (c - 1, [2, 3])))
                    fill_groups[c].append(("store", (c - 1,)))

            # ---- prologue projections: Q/K window 0 + V tiles 0,1 ----
            if stage >= 2:
                for which in range(2):
                    for t in range(NK):
                        for hf in range(2):
                            emit_qk(t, which, 0, half=hf)
                emit_v(0)
                emit_v(1)
            prologue_copies = False

            # ---- main loop: banded attention + interleaved proj + out-proj ----
            otts = [None] * NCH

            yps = {}

            def emit_outproj_mms(ch, ts):
                if ch not in yps:
                    yps[ch] = ps.tile([128, 512], f32, name=f"yp{ch}", tag="big", bufs=2)
                for t in ts:
                    mm(
                        yps[ch][:],
                        otts[ch][:, t * 128 : (t + 1) * 128],
                        wos[t][:],
                        start=(t == 0),
                        stop=(t == NK - 1),
                    )

            def emit_outproj_store(ch, split=False):
                yp = yps[ch]
                ysb = pers.tile([128, D], f32, name=f"ysb{ch}", tag="ysb", bufs=2)
                if split:  # halve the tail chain: DMA half 0 while half 1 adds
                    for hf in range(2):
                        cs = slice(256 * hf, 256 * (hf + 1))
                        nc.vector.tensor_add(
                            out=ysb[:, cs], in0=yp[:, cs], in1=boutb[:, cs]
                        )
                        eng = nc.sync if hf == 0 else nc.scalar
                        eng.dma_start(
                            out=y_d[ch * 128 : (ch + 1) * 128, cs], in_=ysb[:, cs]
                        )
                else:
                    nc.vector.tensor_add(out=ysb[:], in0=yp[:], in1=boutb[:])
                    nc.sync.dma_start(
                        out=y_d[ch * 128 : (ch + 1) * 128, :], in_=ysb[:]
                    )

            def emit_outproj(ch):
                emit_outproj_mms(ch, range(NK))
                emit_outproj_store(ch)

            for ch in range(n_ch if stage >= 3 else 0):
                fills = fill_groups[ch]
                fi = 0

                def fill(n):
                    nonlocal fi
                    for _ in range(min(n, len(fills) - fi)):
                        fn, args = fills[fi]
                        if fn is None:
                            emit_outproj_mms(*args)
                        elif fn == "store":
                            emit_outproj_store(*args)
                        else:
                            fn(*args)
                        fi += 1

                mask = Mfirst if ch == 0 else (Mlast if ch == NCH - 1 else Mmid)
                # scores in t-pair groups: one [128,1024] psum tile spans two
                # banks; all p0=0 blocks fill bank A, p0=64 blocks bank B (the
                # PE wedges if the operand partition offset switches while
                # targeting the same PSUM bank).
                pts = []
                for pg in range(2):
                    scp = ps.tile(
                        [128, 1024], f32, name=f"sc{ch}_{pg}", tag="sc", bufs=2
                    )
                    for hh in range(2):
                        p0 = 64 * hh
                        for ti in range(2):
                            t = 2 * pg + ti
                            for r in range(2):
                                blk = 4 * hh + 2 * ti + r
                                mm(
                                    scp[:, blk * 128 : (blk + 1) * 128],
                                    KT[t][
                                        p0 : p0 + 64,
                                        128 * (ch + r) : 128 * (ch + r + 1),
                                    ],
                                    QT[t][p0 : p0 + 64, 128 * ch : 128 * (ch + 1)],
                                    start=True,
                                    stop=True,
                                )
                    if stage == 31:
                        continue
                    pt = pers.tile(
                        [128, 1024], bf16, name=f"pt{ch}_{pg}", tag="pt", bufs=3
                    )
                    nc.scalar.activation(out=pt[:], in_=scp[:], func=Exp, scale=0.125)
                    if stage == 32:
                        pts.append(pt)
                        continue
                    nc.gpsimd.tensor_mul(out=pt[:, 0:512], in0=pt[:, 0:512], in1=mask[:])
                    nc.gpsimd.tensor_mul(
                        out=pt[:, 512:1024], in0=pt[:, 512:1024], in1=mask[:]
                    )
                    pts.append(pt)

                fill(1)
                if stage < 4 or stage in (31, 32):
                    continue
                opc = ps.tile([128, 512], f32, name=f"op{ch}", tag="op", bufs=1)
                dnc = ps.tile([128, 8], f32, name=f"dn{ch}", tag="dn", bufs=1)
                for t in range(NK):
                    pt = pts[t // 2]
                    for hh in range(2):
                        h = 2 * t + hh
                        for r in range(2):
                            b = 4 * hh + 2 * (t % 2) + r
                            blk = pt[:, b * 128 : (b + 1) * 128]
                            mm(
                                dnc[:, h : h + 1], blk, onesc[:],
                                start=(r == 0), stop=(r == 1),
                            )
                            mm(
                                opc[:, 64 * h : 64 * (h + 1)],
                                blk,
                                Vs[ch + r][:, 64 * h : 64 * (h + 1)],
                                start=(r == 0),
                                stop=(r == 1),
                            )
                    if t < NK - 1:
                        fill(1)

                # normalization: per-query reciprocal, broadcast along d, mult
                if stage < 5:
                    continue
                otq = pers.tile([128, 512], bf16, name=f"otq{ch}", tag="otq", bufs=2)
                ott = pers.tile([128, 512], bf16, name=f"ott{ch}", tag="ott", bufs=3)
                otts[ch] = ott
                if ch >= NCH - 3 and n_ch == NCH:
                    # tail chunks: half-granular norm + transpose so the
                    # consumer out-projections unblock two PV-groups earlier
                    for hf in range(2):
                        cs = slice(256 * hf, 256 * (hf + 1))
                        rrh = pers.tile(
                            [128, 4], f32, name=f"rr{ch}_{hf}", tag="rrh", bufs=2
                        )
                        nc.vector.reciprocal(
                            out=rrh[:], in_=dnc[:, 4 * hf : 4 * (hf + 1)]
                        )
                        rbh = pers.tile(
                            [128, 256], f32, name=f"rb{ch}_{hf}", tag="rbh", bufs=2
                        )
                        rr_ap = rrh[:]
                        rr_b = bass.AP(
                            tensor=rr_ap.tensor,
                            offset=rr_ap.offset,
                            ap=[list(rr_ap.ap[0]), [1, 4], [0, 64]],
                        )
                        nc.gpsimd.tensor_copy(
                            out=rbh[:].rearrange("p (h e) -> p h e", e=64), in_=rr_b
                        )
                        nc.vector.tensor_mul(
                            out=otq[:, cs], in0=opc[:, cs], in1=rbh[:]
                        )
                        if stage < 6:
                            continue
                        if ch != NCH - 1:
                            nc.sync.dma_start_transpose(
                                out=ott[:, cs].rearrange("p (t q) -> p t q", q=128),
                                in_=otq[:, cs],
                            )
                    if stage < 6:
                        continue
                    if ch == NCH - 1:
                        # final chunk: PE transpose + Act copy beats the DMA
                        # transpose's ~1.7us fixed latency; copies on Act so
                        # they don't serialize behind DVE's norm work
                        for t in range(NK):
                            tp = ps.tile(
                                [128, 128], bf16, name=f"tp{ch}_{t}",
                                tag="big", bufs=2,
                            )
                            nc.tensor.transpose(
                                out=tp[:],
                                in_=otq[:, t * 128 : (t + 1) * 128],
                                identity=ident[:],
                            )
                            nc.scalar.copy(
                                out=ott[:, t * 128 : (t + 1) * 128], in_=tp[:]
                            )
                else:
                    rr = pers.tile([128, 8], f32, name=f"rr{ch}", tag="rr", bufs=2)
                    nc.vector.reciprocal(out=rr[:], in_=dnc[:])
                    rb = pers.tile([128, 512], f32, name=f"rb{ch}", tag="rb", bufs=2)
                    rr_ap = rr[:]
                    rr_b = bass.AP(
                        tensor=rr_ap.tensor,
                        offset=rr_ap.offset,
                        ap=[list(rr_ap.ap[0]), [1, 8], [0, 64]],
                    )
                    nc.gpsimd.tensor_copy(
                        out=rb[:].rearrange("p (h e) -> p h e", e=64), in_=rr_b
                    )
                    nc.vector.tensor_mul(out=otq[:], in0=opc[:], in1=rb[:])
                    if stage < 6:
                        continue
                    # batched transpose to d-major: ott[d, t, q] = otq[q, t, d]
                    nc.sync.dma_start_transpose(
                        out=ott[:].rearrange("p (t q) -> p t q", q=128), in_=otq[:]
                    )
                fill(len(fills))
                if stage < 7:
                    continue
                if 1 <= ch and (ch <= 12 or n_ch < NCH):
                    emit_outproj(ch - 1)
            if stage >= 7 and stage not in (31, 32):
                if n_ch == NCH:
                    # final store: fold bout in as a rank-1 PSUM accumulation
                    # and DMA straight from PSUM (skips the ysb add+copy)
                    lc = n_ch - 1
                    emit_outproj_mms(lc, range(NK - 1))
                    mm(
                        yps[lc][:],
                        otts[lc][:, 384:512],
                        wos[NK - 1][:],
                        start=False,
                        stop=False,
                    )
                    mm(yps[lc][:], ones_row[:], bout_row[:], start=False, stop=True)
                    ysbl = pers.tile([128, D], f32, name="ysb_last", tag="ysb", bufs=2)
                    for hf in range(2):
                        cs = slice(256 * hf, 256 * (hf + 1))
                        nc.scalar.copy(out=ysbl[:, cs], in_=yps[lc][:, cs])
                        eng = nc.sync if hf == 0 else nc.scalar
                        eng.dma_start(
                            out=y_d[lc * 128 : (lc + 1) * 128, cs], in_=ysbl[:, cs]
                        )
                else:
                    emit_outproj(n_ch - 1)

    nc.compile()
    return nc


def get_nc():
    if "nc" not in _NC_CACHE:
        _NC_CACHE["nc"] = _build_nc()
    return _NC_CACHE["nc"]


def make_core_inputs(x, Wqkv, Wout, bout):
    """Host-side shard + layout prep: per-core transposed bf16 views."""
    from concourse import mybir

    bf16 = mybir.dt.np(mybir.dt.bfloat16)
    x = np.asarray(x, dtype=np.float32)
    wqkvT = np.ascontiguousarray(np.asarray(Wqkv, dtype=np.float32).T).astype(bf16)
    woutT = np.ascontiguousarray(np.asarray(Wout, dtype=np.float32).T).astype(bf16)
    bout = np.ascontiguousarray(np.asarray(bout, dtype=np.float32))
    in_maps = []
    for b in range(B):
        in_maps.append(
            {
                "xT": np.ascontiguousarray(x[:, b, :].T).astype(bf16),
                "wqkvT": wqkvT,
                "woutT": woutT,
                "bout": bout,
            }
        )
    return in_maps


def kernel(x, Wqkv, Wout, bout):
    from concourse.bass_utils import run_bass_kernel_spmd

    nc = get_nc()
    in_maps = make_core_inputs(x, Wqkv, Wout, bout)
    res = run_bass_kernel_spmd(nc, in_maps, core_ids=list(range(B)))
    out = np.empty((L, B, D), dtype=np.float32)
    for b in range(B):
        out[:, b, :] = res.results[b]["y"]
    return out
